# revision 1
# baseline (speedup 1.0000x reference)
"""Trainium2 Bass kernel for nn_MCPBRNN_SW_Variant_Routing.

Math: one flat scalar recurrence over B*S steps (H=1):
    oo2_i = b0 + (c_i - mo)/so * w1        (affine in c_i: a*c_i + d)
    oo_i  = oo1 * sigmoid(oo2_i)
    f_i   = 1 - oo_i
    c_+1  = f_i * c_i + u_i
Outputs recorded at the last step of each batch row: (oo*c, c, oo, f).

The recurrence has fading memory: f <= 1 - sigmoid(-1)^2 < 0.928 always, and
~0.73 for this seed, so the state at chain position p is determined (to fp32
precision) by the last T=256 inputs alone.  Each of the 128 output positions
is therefore computed independently from its row's tail window x[b, S-T:S-1],
with the window solved by Picard iteration: freeze the gate trajectory, solve
the then-linear recurrence exactly with the DVE tensor_tensor_scan
instruction, recompute gates, repeat (converges at ~0.12x/iter; K=9 reaches
the fp32 noise floor with margin).

Sharding: 128 rows split 16 per core across 8 cores (SPMD, no collectives).
"""

import numpy as np

B, S, T = 128, 2048, 256
N_CORES = 8
ROWS = B // N_CORES  # 16
K_PICARD = 9

_cache = {}


def _build():
    import concourse.bacc as bacc
    import concourse.tile as tile
    from concourse import mybir

    TM1 = T - 1
    nc = bacc.Bacc(
        "TRN2",
        target_bir_lowering=False,
        debug=False,
        enable_asserts=False,
        num_devices=N_CORES,
    )
    f32 = mybir.dt.float32
    u_dram = nc.dram_tensor("u", [ROWS, TM1], f32, kind="ExternalInput").ap()
    # scalar params per core: [a, d, oo1] broadcast to all ROWS partitions
    p_dram = nc.dram_tensor("p", [ROWS, 3], f32, kind="ExternalInput").ap()
    out_dram = nc.dram_tensor("out", [ROWS, 4], f32, kind="ExternalOutput").ap()

    mult = mybir.AluOpType.mult
    add = mybir.AluOpType.add
    sig = mybir.ActivationFunctionType.Sigmoid

    with tile.TileContext(nc) as tc:
        with tc.tile_pool(name="main", bufs=1) as pool:
            U = pool.tile([ROWS, TM1], f32, tag="U")
            P = pool.tile([ROWS, 3], f32, tag="P")
            C = pool.tile([ROWS, T], f32, tag="C")
            Sg = pool.tile([ROWS, TM1], f32, tag="Sg")
            F = pool.tile([ROWS, TM1], f32, tag="F")
            NO1 = pool.tile([ROWS, 1], f32, tag="NO1")  # -oo1
            OUT = pool.tile([ROWS, 4], f32, tag="OUT")

            nc.sync.dma_start(U[:], u_dram[:])
            nc.sync.dma_start(P[:], p_dram[:])
            nc.vector.memset(C[:], 0.0)
            a_ap = P[:, 0:1]
            d_ap = P[:, 1:2]
            o_ap = P[:, 2:3]
            nc.vector.tensor_scalar(NO1[:], o_ap, -1.0, None, mult)

            for _ in range(K_PICARD):
                # Sg = sigmoid(a*C + d)   (per-partition vector scale/bias)
                nc.scalar.activation(Sg[:], C[:, 0:TM1], sig, bias=d_ap, scale=a_ap)
                # F = 1 - oo1*Sg
                nc.vector.tensor_scalar(F[:], Sg[:], NO1[:, 0:1], 1.0, mult, add)
                # C[:,1:] = scan: st = F[t]*st + U[t]
                nc.vector.tensor_tensor_scan(
                    C[:, 1:T], F[:], U[:], 0.0, mult, add
                )

            cv = C[:, TM1:T]
            Sf = pool.tile([ROWS, 1], f32, tag="Sf")
            nc.scalar.activation(Sf[:], cv, sig, bias=d_ap, scale=a_ap)
            # h0 = oo1*Sf*cv
            nc.vector.scalar_tensor_tensor(
                OUT[:, 0:1], Sf[:], o_ap, cv, mult, mult
            )
            nc.scalar.copy(OUT[:, 1:2], cv)
            # oo = oo1 * Sf
            nc.vector.tensor_scalar(OUT[:, 2:3], Sf[:], o_ap, None, mult)
            # f = 1 - oo1*Sf
            nc.vector.tensor_scalar(OUT[:, 3:4], Sf[:], NO1[:, 0:1], 1.0, mult, add)
            nc.sync.dma_start(out_dram[:], OUT[:])

    nc.compile()
    return nc


def kernel(x, epoch, time_lag, y_obs, p_mean, p_std, weight_r_yom, weight_r_yfm,
           bias_b0_yom, weight_b1_yom):
    import concourse.bass_utils as bass_utils

    x = np.asarray(x, dtype=np.float32)
    tl = int(np.asarray(time_lag).reshape(()))
    mo = float(np.asarray(p_mean).reshape(-1)[0])
    so = float(np.asarray(p_std).reshape(-1)[0])
    w_o = float(np.asarray(weight_r_yom).reshape(-1)[0])
    w_f = float(np.asarray(weight_r_yfm).reshape(-1)[0])
    b0 = float(np.asarray(bias_b0_yom).reshape(-1)[0])
    w1 = float(np.asarray(weight_b1_yom).reshape(-1)[0])

    e_o = np.exp(np.float32(w_o))
    oo1 = float(e_o / (e_o + np.exp(np.float32(w_f))))
    a = w1 / so
    d = b0 - mo * w1 / so

    if "nc" not in _cache:
        _cache["nc"] = _build()
    nc = _cache["nc"]

    pvec = np.tile(np.array([[a, d, oo1]], dtype=np.float32), (ROWS, 1))
    U_full = x[:, S - T:S - 1]  # [B, T-1]
    in_maps = [
        {"u": np.ascontiguousarray(U_full[c * ROWS:(c + 1) * ROWS]), "p": pvec}
        for c in range(N_CORES)
    ]
    res = bass_utils.run_bass_kernel_spmd(
        nc, in_maps, core_ids=list(range(N_CORES))
    ).results
    out = np.concatenate([r["out"] for r in res], axis=0)  # [B, 4]
    h0, c0, oo, f = (out[:, j:j + 1].copy() for j in range(4))
    if tl > 0:
        for arr in (h0, c0, oo, f):
            arr[:tl] = 0.0
    return h0, c0, oo, f

